# revision 17
# baseline (speedup 1.0000x reference)
"""Trainium2 Bass kernel for nn_CustomModel_1159641170247.

Yield-stress material model on (50,6) inputs:
    param_deltaH = 0.1 + 4.9*sigmoid(raw)   (7,6) -> gathered to (50,6)
    param_KHP    = exp(raw)                 (7,)  -> gathered to (50,)
    W            = symmetric 6x6 from 21 upper-tri params, 0.1+exp
    A            = LSR @ W
    therm        = KB*T*ln(1e4/Srate) / deltaH
    tau          = sum(A*(1 - therm^(2/3)), axis=1)
    out          = tau*2.733 + KHP*GrainSize^-0.5

Strategy: the whole problem is ~2 KB, latency-bound. One tiny single-core
program, replicated on all 8 cores (per sharding hint). Everything is
host-packed into ONE input tensor -> ONE input DMA, so every consumer has a
single DMA tick to wait on. Design rule: at most one cross-engine wait per
instruction (the TensorScalar encoding cannot hold more).

Tricks:
  * The constant-index gather (GROUP_IDX) runs FIRST as a one-hot matmul on
    the RAW params (gather commutes with elementwise), so the rhs is
    DMA-written only; sigmoid/exp run post-gather on [50,*] tiles.
  * W's `0.1 + exp(w)` folds into one Exp: lhsT stacks LSR^T twice (K=12)
    and rhs rows 6:12 hold ln(0.1), so exp() yields the 0.1 addend and the
    PSUM accumulate adds it -- rhs is single-writer (ACT).

    O[50, 0:7]  = [S](50x7)    @ raw[deltaH | KHP](7x7)
    O[50, 7:13] = [LSR|LSR]    @ [exp(w_sym); exp(ln 0.1)](12x6)
"""

import numpy as np

import concourse.bass as bass
import concourse.mybir as mybir
import concourse.tile as tile
from concourse import bass_utils
from concourse.tile_scheduler import PROC_NAME_TO_IDX

_IDX_TO_PROC = {v: k for k, v in PROC_NAME_TO_IDX.items()}


class _SplitDrainTileContext(tile.TileContext):
    """TileContext whose tail drain issues one standalone wait per semaphore.

    The stock epilogue attaches every final sem wait to a single SP Drain
    instruction; with >3 active procs (engines + DMA lanes) that overflows
    the Drain encoding's sync-wait slots and walrus refuses to codegen.
    Standalone wait_ge instructions have no such limit.
    """

    def _drain_and_barrier(self, tick_clock, wait_clock):
        nc = self.nc
        gc = tick_clock.global_clock.copy()
        for proc_idx, sem in sorted(self.sems.allocated().items()):
            tick = gc.advance(proc_idx) - 1
            if tick <= 0:
                continue
            mult = 16 if _IDX_TO_PROC[proc_idx].startswith("DMA") else 1
            nc.sync.wait_ge(sem, tick * mult)
        nc.sync.drain()
        nc.all_engine_barrier()
        popped = nc._tile_sem_poison_stack.pop()
        assert popped is self._sem_poison
        nc.clear_and_free_semaphores(list(self.sems.allocated().values()))
        nc.all_engine_barrier()

F32 = mybir.dt.float32
AF = mybir.ActivationFunctionType
ALU = mybir.AluOpType

KB = 8.62e-05
PARAM_M = 2.733
N_CORES = 8

# --- compile-time constants of the model (from the reference source) ---
GROUP_COUNTS = np.array([1, 2, 8, 7, 6, 9, 17])
GROUP_IDX = np.repeat(np.arange(7), GROUP_COUNTS)  # (50,)
_S_T = (GROUP_IDX[None, :] == np.arange(7)[:, None]).astype(np.float32)  # (7,50)
_iu, _ju = np.triu_indices(6)
_SYM = np.zeros((6, 6), dtype=np.int64)
_SYM[_iu, _ju] = np.arange(21)
_SYM[_ju, _iu] = np.arange(21)

# mega-pack column layout (50 partitions x 116 f32)
_C_PAR = 0      # cols 0:7   rows 0:7  raw [deltaH(6) | KHP]
_C_W = 7        # cols 7:13  rows 0:12 [w_sym(6 rows); ln(0.1)(6 rows)]
_C_SEL = 13     # cols 13:63 rows 0:7  S^T one-hot selection
_C_LSR = 63     # cols 63:113 rows 0:12 LSR^T stacked twice
_C_T = 113      # Temp
_C_S = 114      # Srate
_C_G = 115      # GrainSize
_C_TOT = 116


def build_nc() -> bass.Bass:
    nc = bass.Bass(trn_type="TRN2")

    all_in = nc.dram_tensor("all_in", (50, _C_TOT), F32, kind="ExternalInput")
    y_out = nc.dram_tensor("yield_out", (50, 1), F32, kind="ExternalOutput")

    with _SplitDrainTileContext(nc) as tc:
        with (
            tc.tile_pool(name="sb", bufs=1) as sb,
            tc.tile_pool(name="ps", bufs=1, space="PSUM") as ps,
        ):
            T = sb.tile([50, _C_TOT], F32)
            nc.sync.dma_start(out=T[:], in_=all_in[:, :])

            # DVE warm-up: make the vector engine observe the input-DMA tick
            # now, so no later DVE instruction needs a (DMA + compute) double
            # wait -- the TensorScalar/STT encodings hold only one.
            warm = sb.tile([1, 1], F32)
            i_warm = nc.vector.tensor_copy(warm[:], T[0:1, 0:1])

            # W blocks: exp(w_sym) and exp(ln 0.1)=0.1   [waits: DMA]
            E12 = sb.tile([12, 6], F32)
            nc.scalar.activation(E12[:], T[0:12, _C_W:_C_W + 6], AF.Exp)

            # param gather via one-hot (raw params! elementwise comes after)
            O = ps.tile([50, 13], F32)
            nc.tensor.matmul(  # [waits: DMA]
                out=O[:, 0:7],
                lhsT=T[0:7, _C_SEL:_C_SEL + 50],
                rhs=T[0:7, 0:7],
                start=True,
                stop=True,
            )
            # A = LSR@exp(w) + LSR@0.1   [waits: ACT(E12)]
            nc.tensor.matmul(
                out=O[:, 7:13],
                lhsT=T[0:12, _C_LSR:_C_LSR + 50],
                rhs=E12[:],
                start=True,
                stop=True,
            )

            # row scalars (ACT, only need the DMA tick already observed)
            t = sb.tile([50, 1], F32)
            nc.scalar.activation(t[:], T[:, _C_S:_C_S + 1], AF.Ln, scale=1e-4)
            sq = sb.tile([50, 1], F32)
            nc.scalar.activation(sq[:], T[:, _C_G:_C_G + 1], AF.Sqrt)

            # post-gather param constraints  [sig waits: PE]
            sig = sb.tile([50, 6], F32)
            nc.scalar.activation(sig[:], O[:, 0:6], AF.Sigmoid)
            kexp = sb.tile([50, 1], F32)
            nc.scalar.activation(kexp[:], O[:, 6:7], AF.Exp)
            D = sb.tile([50, 6], F32)
            nc.scalar.activation(D[:], sig[:], AF.Copy, bias=0.1, scale=4.9)

            # q = KB * Temp * ln(1e4/Srate)   [waits: ACT(t); DMA via warm-up]
            q = sb.tile([50, 1], F32)
            i_q = nc.vector.scalar_tensor_tensor(
                q[:], in0=t[:], scalar=-KB, in1=T[:, _C_T:_C_T + 1],
                op0=ALU.mult, op1=ALU.mult,
            )
            tile.add_dep_helper(i_q.ins, i_warm.ins, sync=False)
            rg = sb.tile([50, 1], F32)
            nc.vector.reciprocal(rg[:], sq[:])

            # A leaves PSUM via ACT (which already observed the PE tick), so
            # no DVE instruction ever reads PSUM -> no DVE wait on PE.
            Acp = sb.tile([50, 6], F32)
            i_acp = nc.scalar.activation(Acp[:], O[:, 7:13], AF.Copy)

            # therm = q / deltaH   [rcpD waits: ACT(D), covers kexp]
            rcpD = sb.tile([50, 6], F32)
            nc.vector.reciprocal(rcpD[:], D[:])
            therm = sb.tile([50, 6], F32)
            i_therm = nc.vector.tensor_scalar(
                therm[:], rcpD[:], q[:], None, op0=ALU.mult
            )
            khp = sb.tile([50, 1], F32)
            i_khp = nc.vector.tensor_mul(khp[:], kexp[:], rg[:])
            tile.add_dep_helper(i_khp.ins, i_therm.ins, sync=False)

            # pw = therm ** (2/3)   [waits: DVE(therm)]
            lnth = sb.tile([50, 6], F32)
            i_lnth = nc.scalar.activation(lnth[:], therm[:], AF.Ln)
            tile.add_dep_helper(i_lnth.ins, i_acp.ins, sync=False)
            pw = sb.tile([50, 6], F32)
            nc.scalar.activation(pw[:], lnth[:], AF.Exp, scale=float(2.0 / 3.0))

            # negtau = sum((pw-1)*A, axis=1)
            pm1 = sb.tile([50, 6], F32)
            nc.vector.tensor_scalar(  # [waits: ACT(pw)]
                pm1[:], pw[:], 1.0, None, op0=ALU.subtract
            )
            junk = sb.tile([50, 6], F32)
            negtau = sb.tile([50, 1], F32)
            nc.vector.scalar_tensor_tensor(  # [waits: DVE self only]
                junk[:], in0=pm1[:], scalar=0.0, in1=Acp[:],
                op0=ALU.add, op1=ALU.mult, accum_out=negtau[:],
            )
            # y = negtau*(-M) + khp
            y = sb.tile([50, 1], F32)
            nc.vector.tensor_scalar(
                y[:], negtau[:], -PARAM_M, khp[:], op0=ALU.mult, op1=ALU.add
            )

            nc.sync.dma_start(out=y_out[:, :], in_=y[:])  # [waits: DVE]

    return nc


def pack_inputs(inputs: dict) -> dict:
    """Host-side layout prep (pure data movement, no arithmetic)."""
    LSR = np.ascontiguousarray(inputs["LSR_input"], dtype=np.float32)
    T = np.asarray(inputs["Temp_input"], dtype=np.float32)
    S = np.asarray(inputs["Srate_input"], dtype=np.float32)
    G = np.asarray(inputs["GrainSize_input"], dtype=np.float32)
    w21 = np.asarray(inputs["sym_weight_raw"], dtype=np.float32)
    rdH = np.asarray(inputs["raw_param_deltaH"], dtype=np.float32)
    rK = np.asarray(inputs["raw_param_KHP"], dtype=np.float32)

    a = np.zeros((50, _C_TOT), np.float32)
    a[0:7, 0:6] = rdH
    a[0:7, 6] = rK
    a[0:6, _C_W:_C_W + 6] = w21[_SYM]  # symmetric, row/col layout identical
    a[6:12, _C_W:_C_W + 6] = np.float32(np.log(np.float32(0.1)))
    a[0:7, _C_SEL:_C_SEL + 50] = _S_T
    a[0:6, _C_LSR:_C_LSR + 50] = LSR.T
    a[6:12, _C_LSR:_C_LSR + 50] = LSR.T
    a[:, _C_T] = T
    a[:, _C_S] = S
    a[:, _C_G] = G
    return {"all_in": a}


_NC_CACHE: list = []


def _get_nc() -> bass.Bass:
    if not _NC_CACHE:
        _NC_CACHE.append(build_nc())
    return _NC_CACHE[0]


def run_on_hw(inputs: dict, trace: bool = False) -> bass_utils.BassKernelResults:
    in_map = pack_inputs(inputs)
    nc = _get_nc()
    return bass_utils.run_bass_kernel_spmd(
        nc, [in_map] * N_CORES, core_ids=list(range(N_CORES)), trace=trace
    )


def kernel(**inputs) -> np.ndarray:
    res = run_on_hw(inputs, trace=False)
    return np.asarray(res.results[0]["yield_out"], dtype=np.float32).reshape(50)
